# revision 18
# baseline (speedup 1.0000x reference)
"""Trainium2 Bass kernel for nn_MixtureOfMixers (moe_routing).

Strategy: data-parallel over batch B=32 across 8 NeuronCores (4 samples/core).
Router, expert gather (dynamic-offset DMAs keyed by on-device top-2 indices),
mixing and projections are all batch-local; no collectives. Expert weights are
fetched per (sample, k) with register-offset DMAs so only selected experts'
weights cross HBM.

Self-contained: hardcodes all shapes; host side only shards/transposes inputs
and concatenates outputs (plus the trivial aux-loss reduction over per-core
router probabilities).
"""

import numpy as np
from contextlib import ExitStack

import concourse.bass as bass
import concourse.mybir as mybir
import concourse.tile as tile
from concourse.bass_utils import run_bass_kernel_spmd
from concourse.vector_clock import ScopedClock

# problem shapes
B, N, D, H, E, K = 32, 512, 1024, 8, 8, 2
hd = 128
HID = 128
NCORES = 8
Bc = B // NCORES  # 4 samples per core
P = 128

F32 = mybir.dt.float32
F32R = mybir.dt.float32r
U32 = mybir.dt.uint32
I32 = mybir.dt.int32
AF = mybir.ActivationFunctionType
AX = mybir.AxisListType
ALU = mybir.AluOpType


# --- workaround: this container's walrus rejects Drain instructions carrying
# multiple sem waits ("Too many sync wait commands"). Re-emit the Tile exit as
# one NOP per wait + sem-only barriers (no InstDrain).
def _patched_drain_and_barrier(self, tick_clock, wait_clock):
    nc = self.nc
    collector = nc.sync.nop()
    wait_clock.add_sem_waits(collector.ins, ScopedClock({None: tick_clock.global_clock}))
    si = collector.ins.sync_info
    waits = list(si.on_wait) if si is not None else []
    if len(waits) > 1:
        si.on_wait = waits[:1]
        for w in waits[1:]:
            extra = nc.sync.nop()
            extra.ins.sync_info = type(si)(on_wait=[w], on_update=[])
    nc.all_engine_barrier(sem_only=True)
    assert self.sems is not None
    popped = nc._tile_sem_poison_stack.pop()
    assert popped is self._sem_poison
    nc.clear_and_free_semaphores(list(self.sems.allocated().values()))
    nc.all_engine_barrier(sem_only=True)


tile.TileContext._drain_and_barrier = _patched_drain_and_barrier

_MAX_WAITS = 1  # this walrus build rejects multiple sync-wait commands per instruction


def _split_excess_waits(nc):
    """Move excess sem-waits onto same-engine NOPs inserted just before the
    instruction (waits are AND conditions; engine program order preserved)."""
    n_split = 0
    for fn in nc.m.functions:
        for blk in fn.blocks:
            new_list = []
            for inst in blk.instructions:
                si = inst.sync_info
                waits = list(si.on_wait) if si is not None else []
                if len(waits) > _MAX_WAITS:
                    si.on_wait = waits[-_MAX_WAITS:]
                    extra = waits[: -_MAX_WAITS]
                    for i0 in range(0, len(extra), _MAX_WAITS):
                        n_split += 1
                        new_list.append(
                            mybir.InstNoOp(
                                name=f"{inst.name}-ws{i0}",
                                engine=inst.engine,
                                sync_info=mybir.SyncInfo(
                                    on_wait=extra[i0 : i0 + _MAX_WAITS], on_update=[]
                                ),
                            )
                        )
                new_list.append(inst)
            blk.instructions[:] = new_list
    return n_split


def build_kernel():
    nc = bass.Bass("TRN2", target_bir_lowering=False, debug=False)

    # ---- per-core external inputs (pre-sharded / pre-transposed on host) ----
    x_t = nc.dram_tensor("x_t", [Bc, D, N], F32R, kind="ExternalInput")
    in_wT = nc.dram_tensor("in_wT", [D, D], F32R, kind="ExternalInput")
    in_b = nc.dram_tensor("in_b", [1, D], F32R, kind="ExternalInput")
    out_wT = nc.dram_tensor("out_wT", [D, D], F32R, kind="ExternalInput")
    out_b = nc.dram_tensor("out_b", [1, D], F32R, kind="ExternalInput")
    router_wT = nc.dram_tensor("router_wT", [D, E], F32R, kind="ExternalInput")
    # wall: per (e,h) row-block of 128 partitions x 1920:
    # [w1t' (4x128) | w2t' (512) | b2 (512) | w1c' (128) | b1' (128) | w2c' (128)]
    WF = 1920
    wall = nc.dram_tensor("wall", [E * H * P, WF], F32R, kind="ExternalInput")
    ones_d = nc.dram_tensor("ones", [P, P], F32R, kind="ExternalInput")

    out = nc.dram_tensor("out", [Bc, N, D], F32, kind="ExternalOutput")
    probs_out = nc.dram_tensor("probs_out", [Bc, E], F32, kind="ExternalOutput")

    tw_scratch = nc.dram_tensor("tw_scratch", [Bc, K], F32)  # internal bounce
    ix_scratch = nc.dram_tensor("ix_scratch", [Bc, K], I32)  # internal bounce
    # inline const: hiota[p, h] = h*128 + p (row index of head h, partition p)
    hiota_np = (np.arange(H, dtype=np.int32)[None, :] * P
                + np.arange(P, dtype=np.int32)[:, None])
    hiota_d = nc.inline_tensor(np.ascontiguousarray(hiota_np), "hiota")

    with tile.TileContext(nc) as tc, ExitStack() as ctx:
        const = ctx.enter_context(tc.tile_pool(name="const", bufs=1))
        strm = ctx.enter_context(tc.tile_pool(name="strm", bufs=2))
        perb = ctx.enter_context(tc.tile_pool(name="perb", bufs=2))
        omp = ctx.enter_context(tc.tile_pool(name="omp", bufs=1))
        gath = ctx.enter_context(tc.tile_pool(name="gath", bufs=2))
        mixp = ctx.enter_context(tc.tile_pool(name="mixp", bufs=2))
        lnp = ctx.enter_context(tc.tile_pool(name="lnp", bufs=1))
        rout = ctx.enter_context(tc.tile_pool(name="rout", bufs=2))
        ps_ip = ctx.enter_context(tc.tile_pool(name="ps_ip", bufs=1, space="PSUM"))
        ps_mx = ctx.enter_context(tc.tile_pool(name="ps_mx", bufs=1, space="PSUM"))
        ps_op = ctx.enter_context(tc.tile_pool(name="ps_op", bufs=2, space="PSUM"))

        # ---------------- constants / static weights (ACT HWDGE ring) -------
        inw_sb = const.tile([P, 8, D], F32R)  # [dpart, dchunk, d']
        nc.scalar.dma_start(inw_sb, in_wT.rearrange("(c p) d -> p c d", p=P))
        outw_sb = const.tile([P, H, D], F32R)  # [cpart, h, d']
        nc.scalar.dma_start(outw_sb, out_wT.rearrange("(h p) d -> p h d", p=P))
        rw_sb = const.tile([P, 8, E], F32R)
        nc.scalar.dma_start(rw_sb, router_wT.rearrange("(c p) e -> p c e", p=P))
        inb_row = const.tile([1, D], F32R)
        nc.scalar.dma_start(inb_row, in_b.ap())
        outb_row = const.tile([1, D], F32R)
        nc.scalar.dma_start(outb_row, out_b.ap())

        ones_sb = const.tile([P, P], F32R)
        nc.scalar.dma_start(ones_sb, ones_d.ap())
        ones_row = ones_sb[0:1, :]
        eps_sb = const.tile([P, 1], F32)
        nc.vector.memset(eps_sb, 1e-5)

        twb = const.tile([P, Bc, K], F32)  # partition-broadcast top-2 weights
        hiota = const.tile([P, H], I32)
        nc.scalar.dma_start(hiota, hiota_d.ap())

        # ---------------- per-sample pipeline ----------------
        for b in range(Bc):
            # --- stream x.T blocks: in_proj matmuls + token sums ---
            xp = perb.tile([P, 4, D], F32R, tag="xp")  # [np, ntile, d'] (xn in-place)
            xsp = rout.tile([P, 8, 4], F32, tag="xsp")  # partial sums per (chunk, nt)
            for nt in range(4):
                blk = strm.tile([P, 8, P], F32R, tag="xtblk")  # [dp, dchunk, n]
                nc.scalar.dma_start(
                    blk, x_t[b].rearrange("(c p) n -> p c n", p=P)[:, :, nt * P : (nt + 1) * P]
                )
                ps = ps_ip.tile([P, 2, 512], F32, tag="ip")
                for c in range(8):
                    for dp in range(2):
                        nc.tensor.matmul(
                            ps[:, dp],
                            blk[:, c, :],
                            inw_sb[:, c, dp * 512 : (dp + 1) * 512],
                            start=(c == 0), stop=False,
                        )
                    nc.vector.reduce_sum(xsp[:, c, nt : nt + 1], blk[:, c, :], axis=AX.X)
                for dp in range(2):  # fold in_b via K=1 contraction row
                    nc.tensor.matmul(
                        ps[:, dp],
                        ones_row,
                        inb_row[:, dp * 512 : (dp + 1) * 512],
                        start=False, stop=True,
                    )
                nc.scalar.activation(xp[:, nt, :], ps.rearrange("p a b -> p (a b)"), AF.Copy)

            # --- router for sample b ---
            xsum = rout.tile([P, 8, 1], F32R, tag="xsum")
            with nc.allow_low_precision(reason="router logits tolerate f32r rounding"):
                nc.vector.reduce_sum(xsum.rearrange("p c o -> p (c o)"), xsp, axis=AX.X)
            lg_ps = ps_ip.tile([1, E], F32, tag="ip")
            for c in range(8):
                nc.tensor.matmul(
                    lg_ps, xsum[:, c, :], rw_sb[:, c, :],
                    start=(c == 0), stop=(c == 7),
                )
            lg = rout.tile([1, E], F32, tag="lgt")
            nc.vector.tensor_scalar(lg, lg_ps, 1.0 / N, None, op0=ALU.mult)
            lmax = rout.tile([1, 1], F32, tag="lmax")
            nc.vector.reduce_max(lmax, lg, axis=AX.X)
            nc.vector.tensor_scalar(lg, lg, lmax, None, op0=ALU.subtract)
            nc.scalar.activation(lg, lg, AF.Exp)
            lsum = rout.tile([1, 1], F32, tag="lsum")
            nc.vector.reduce_sum(lsum, lg, axis=AX.X)
            lrec = rout.tile([1, 1], F32, tag="lrec")
            nc.vector.reciprocal(lrec, lsum)
            probs = rout.tile([1, E], F32, tag="probs")
            nc.vector.tensor_scalar(probs, lg, lrec, None, op0=ALU.mult)
            nc.sync.dma_start(probs_out.ap()[b : b + 1, :], probs)

            mx = rout.tile([1, 8], F32, tag="mx")
            ix = rout.tile([1, 8], U32, tag="ix")
            nc.vector.max_with_indices(mx, ix, probs)
            twsum = rout.tile([1, 1], F32, tag="twsum")
            nc.vector.tensor_tensor(twsum, mx[:, 0:1], mx[:, 1:2], op=ALU.add)
            twrec = rout.tile([1, 1], F32, tag="twrec")
            nc.vector.reciprocal(twrec, twsum)
            tw = rout.tile([1, K], F32, tag="tw")
            nc.vector.tensor_scalar(tw, mx[:, 0:K], twrec, None, op0=ALU.mult)
            # bounce through DRAM to broadcast across partitions
            nc.sync.dma_start(tw_scratch.ap()[b : b + 1, :], tw)
            nc.gpsimd.dma_start(
                out=twb[:, b],
                in_=bass.AP(tensor=tw_scratch, offset=b * K, ap=[[0, P], [1, K]]),
            )
            # bounce top-2 indices through DRAM -> all partitions, build row idx
            nc.sync.dma_start(ix_scratch.ap()[b : b + 1, :], ix[:, 0:K].bitcast(I32))
            ixb = rout.tile([P, K], I32, tag="ixb")
            nc.gpsimd.dma_start(
                out=ixb,
                in_=bass.AP(tensor=ix_scratch, offset=b * K, ap=[[0, P], [1, K]]),
            )
            # idx_t[p, k, h] = ixb[p,k]*1024 + h*128 + p  (row in wall)
            idx_t = rout.tile([P, K, H], I32, tag="idx_t")
            nc.vector.scalar_tensor_tensor(
                idx_t,
                ixb[:, :, None].to_broadcast([P, K, H]),
                float(H * P),
                hiota[:, None, :].to_broadcast([P, K, H]),
                op0=ALU.mult, op1=ALU.add,
            )


            # --- layernorm over tokens (columns d', partition-replicated) ---
            for half in range(2):
                dsl = slice(half * 512, (half + 1) * 512)
                sums = ps_ip.tile([P, 1024], F32, tag="ip")  # [sum | sumsq]
                for nt in range(4):
                    xs = xp[:, nt, dsl]
                    nc.tensor.matmul(
                        sums[:, 0:512], ones_sb, xs,
                        start=(nt == 0), stop=(nt == 3),
                    )
                    sq = mixp.tile([P, 512], F32R, tag="sq")
                    nc.scalar.activation(sq, xs, AF.Square)
                    nc.tensor.matmul(
                        sums[:, 512:1024], ones_sb, sq,
                        start=(nt == 0), stop=(nt == 3),
                    )
                mu = lnp.tile([P, 512], F32, tag="mu")
                nc.vector.tensor_scalar(mu, sums[:, 0:512], 1.0 / N, None, op0=ALU.mult)
                var = lnp.tile([P, 512], F32, tag="var")
                nc.vector.tensor_scalar(var, sums[:, 512:1024], 1.0 / N, None, op0=ALU.mult)
                musq = lnp.tile([P, 512], F32, tag="musq")
                nc.vector.tensor_tensor(musq, mu, mu, op=ALU.mult)
                nc.vector.tensor_tensor(var, var, musq, op=ALU.subtract)
                rstd = lnp.tile([P, 512], F32, tag="rstd")
                nc.scalar.activation(rstd, var, AF.Sqrt, bias=eps_sb, scale=1.0)
                nc.vector.reciprocal(rstd, rstd)
                for nt in range(4):
                    xs = xp[:, nt, dsl]
                    nc.vector.tensor_tensor(xs, xs, mu, op=ALU.subtract)
                    nc.vector.tensor_tensor(xs, xs, rstd, op=ALU.mult)

            # --- mixer over heads ---
            om = omp.tile([P, H, N], F32R, tag="om")  # out_mix [c, h, n]
            for h in range(H):
                wh = gath.tile([P, K, 1920], F32R, tag="wh")
                for k in range(K):
                    nc.gpsimd.indirect_dma_start(
                        out=wh[:, k],
                        out_offset=None,
                        in_=wall.ap(),
                        in_offset=bass.IndirectOffsetOnAxis(
                            ap=idx_t[:, k, h : h + 1], axis=0
                        ),
                    )

                # stage 1: h1[d, (k,e')] = sum_n xn[n,d] * w1t[(k),n,e']
                s1 = ps_mx.tile([P, K, HID], F32, tag="s12")
                for nchunk in range(4):
                    nc.tensor.matmul(
                        s1,
                        xp[:, nchunk, h * hd : (h + 1) * hd],
                        wh[:, :, nchunk * P : (nchunk + 1) * P],
                        start=(nchunk == 0), stop=(nchunk == 3),
                    )
                h1 = mixp.tile([P, K, HID], F32R, tag="h1")
                nc.vector.tensor_copy(h1, s1)

                # stage 2 (transposed out): h2T[e', c] = sum_d h1[d,e'] w1cT[d,c]
                s2 = ps_mx.tile([P, K, hd], F32, tag="s12")
                for k in range(K):
                    nc.tensor.matmul(
                        s2[:, k], h1[:, k], wh[:, k, 1536:1664],
                        start=True, stop=True,
                    )
                h2 = mixp.tile([P, K, hd], F32R, tag="h2")
                nc.vector.tensor_tensor(h2, s2, wh[:, :, 1664:1792], op=ALU.add)
                nc.scalar.activation(h2, h2, AF.Gelu_apprx_tanh)

                # stage 3: o[c, n] = sum_e' h2[e',c] w2tT[e',n]  (tw folded in)
                s3 = ps_mx.tile([P, K, N], F32, tag="s3")
                for k in range(K):
                    nc.tensor.matmul(
                        s3[:, k], h2[:, k], wh[:, k, 512:1024],
                        start=True, stop=True,
                    )
                osc = mixp.tile([P, K, N], F32R, tag="osc")
                nc.scalar.activation(osc[:, 0], s3[:, 0], AF.Copy, scale=twb[:, b, 0:1])
                nc.vector.tensor_scalar(osc[:, 1], s3[:, 1], twb[:, b, 1:2], None, op0=ALU.mult)

                # stage 4: om[c, n] = sum_k w2c[c,d] osc[d,n]  (+ tw-weighted b2)
                s4 = ps_mx.tile([P, N], F32, tag="s4")
                for k in range(K):
                    nc.tensor.matmul(
                        s4, wh[:, k, 1792:1920], osc[:, k],
                        start=(k == 0), stop=(k == 1),
                    )
                nc.vector.scalar_tensor_tensor(
                    om[:, h], wh[:, 0, 1024:1536], twb[:, b, 0:1], s4,
                    op0=ALU.mult, op1=ALU.add,
                )
                nc.vector.scalar_tensor_tensor(
                    om[:, h], wh[:, 1, 1024:1536], twb[:, b, 1:2], om[:, h],
                    op0=ALU.mult, op1=ALU.add,
                )

            # --- out_proj: out[n, d'] = sum_{h,c} om[c,(h),n-tile] out_wT ---
            for nt in range(4):
                for dp in range(2):
                    ps = ps_op.tile([P, 512], F32, tag="op")
                    for h in range(H):
                        nc.tensor.matmul(
                            ps,
                            om[:, h, nt * P : (nt + 1) * P],
                            outw_sb[:, h, dp * 512 : (dp + 1) * 512],
                            start=(h == 0), stop=False,
                        )
                    nc.tensor.matmul(  # fold out_b via K=1 contraction row
                        ps, ones_row, outb_row[:, dp * 512 : (dp + 1) * 512],
                        start=False, stop=True,
                    )
                    osb = mixp.tile([P, 512], F32, tag="osb")
                    nc.scalar.activation(osb, ps, AF.Copy)
                    nc.scalar.dma_start(
                        out[b, nt * P : (nt + 1) * P, dp * 512 : (dp + 1) * 512], osb
                    )

    _split_excess_waits(nc)
    return nc


_NC_CACHE = None


def _get_nc():
    global _NC_CACHE
    if _NC_CACHE is None:
        _NC_CACHE = build_kernel()
    return _NC_CACHE


def _prep_core_inputs(inputs):
    """host-side shard + layout prep; returns list of per-core input maps"""
    x = np.ascontiguousarray(np.asarray(inputs["x"], dtype=np.float32))
    f = lambda k: np.asarray(inputs[k], dtype=np.float32)
    in_wT = np.ascontiguousarray(f("in_w").T)
    out_wT = np.ascontiguousarray(f("out_w").T)
    router_wT = np.ascontiguousarray(f("router_w").T)
    # wall[(e,h,p), 0:1920] = [w1t' | w2t' | b2 | w1c' | b1' | w2c']
    wall = np.empty((E, H, P, 1920), dtype=np.float32)
    w1t = f("w1t")  # (E,H,HID,N)
    # w1t' block: [np, (c, e')]: w1t[e,h].T (N,HID) -> (4,128,HID) -> (np, c, e')
    wall[:, :, :, 0:512] = (
        f("w1t").transpose(0, 1, 3, 2).reshape(E, H, 4, P, HID).transpose(0, 1, 3, 2, 4).reshape(E, H, P, 512)
    )
    wall[:, :, :, 512:1024] = f("w2t").transpose(0, 1, 3, 2)  # (E,H,HID,N)
    wall[:, :, :, 1024:1536] = f("b2")  # (E,H,hd,N)
    wall[:, :, :, 1536:1664] = f("w1c").transpose(0, 1, 3, 2)
    wall[:, :, :, 1664:1792] = f("b1").transpose(0, 1, 3, 2)
    wall[:, :, :, 1792:1920] = f("w2c").transpose(0, 1, 3, 2)
    wall = np.ascontiguousarray(wall.reshape(E * H * P, 1920))
    in_b = np.ascontiguousarray(f("in_b").reshape(1, D))
    out_b = np.ascontiguousarray(f("out_b").reshape(1, D))

    maps = []
    for c in range(NCORES):
        xs = x[c * Bc : (c + 1) * Bc]  # (Bc, N, D)
        x_tc = np.ascontiguousarray(xs.transpose(0, 2, 1))  # (Bc, D, N)
        maps.append(
            {
                "x_t": x_tc,
                "in_wT": in_wT,
                "in_b": in_b,
                "out_wT": out_wT,
                "out_b": out_b,
                "router_wT": router_wT,
                "ones": np.ones((P, P), dtype=np.float32),
                "wall": wall,
            }
        )
    return maps


def run_on_device(inputs, trace=False, **kw):
    nc = _get_nc()
    in_maps = _prep_core_inputs(inputs)
    res = run_bass_kernel_spmd(nc, in_maps, core_ids=list(range(NCORES)), trace=trace, **kw)
    return res


def kernel(**inputs):
    res = run_on_device(inputs)
    outs = [r["out"] for r in res.results]  # each (Bc, N, D)
    full = np.concatenate(outs, axis=0)
    probs = np.concatenate([r["probs_out"] for r in res.results], axis=0)  # (B, E)
    # aux loss: E * sum_e mean_b(probs)_e * mean_b(onehot(top1))_e
    top1 = probs.argmax(axis=1)
    mask = np.zeros((B, E), dtype=np.float32)
    mask[np.arange(B), top1] = 1.0
    aux = np.float32(E * np.sum(probs.mean(axis=0) * mask.mean(axis=0)))
    return (full, aux)
